# revision 1
# baseline (speedup 1.0000x reference)
"""Per-pixel adaptive 5x5 conv (KPN) for Trainium2, 8-core data parallel.

out[g,h,w] = sum_{i,j} core[g,5i+j,h,w] * frames_pad[g,h+i-2,w+j-2]
with g = flattened (B,N) = 16 image planes; 2 planes per NeuronCore.

Host prep builds DMA-friendly layouts (all fp16):
  fin [2, 128, 5*2*4*518]: per partition p: [i-shift:5][parity:2][blk:4][518]
     fprep[img,p,i,par,blk,c] = Fpad[img, blk*128+p+i, (1-par)+c]
     parity copies keep every tap's 512-col slice 4-byte aligned so the
     DVE 2x fp16 mode engages for all 25 (i,j) taps.
  win [2, 5, 128, 5*4*512]: tap-group-major core weights
     wprep[img,tg,p,k,blk,c] = core[img, 5*tg+k, blk*128+p, c]
On chip per image: 1 frames DMA + 5 weight-group DMAs; 25 taps of
mul+add at FD=2048 (4 row-blocks fused per op), 19 taps on DVE (fp16
2x mode) and 6 on GpSimd; two accumulator chains merged at the end;
fp16->fp32 cast on the output DMA (SWDGE).
"""

import os
import sys

import numpy as np

for _p in ("/opt/trn_rl_repo",):
    if _p not in sys.path and os.path.isdir(_p):
        sys.path.insert(0, _p)

K = 5
NCORES = 8
IMGS_PER_CORE = 2
H = W = 512
NBLK = 4  # 128-row blocks per image
FCOLS = 518
F_FREE = K * 2 * NBLK * FCOLS  # 20720
W_FREE = K * NBLK * W  # 10240
O_FREE = NBLK * W  # 2048

# gpsimd gets ~1/4 of taps (it runs 2-input elemwise ~2.9x slower than
# DVE fp16-2x): one tap per 5-tap group + one extra.
GP_TAPS = frozenset({4, 9, 14, 19, 23, 24})

_compiled = {}
last_results = None  # BassKernelResults of the most recent run (for test.py)


def _build_nc():
    import concourse.bacc as bacc
    import concourse.mybir as mybir
    from concourse.tile import TileContext

    f16 = mybir.dt.float16
    f32 = mybir.dt.float32

    nc = bacc.Bacc(None, target_bir_lowering=False, debug=False)
    fin = nc.dram_tensor("fin", [IMGS_PER_CORE, 128, F_FREE], f16,
                         kind="ExternalInput")
    win = nc.dram_tensor("win", [IMGS_PER_CORE, K, 128, W_FREE], f16,
                         kind="ExternalInput")
    oout = nc.dram_tensor("oout", [IMGS_PER_CORE, 128, O_FREE], f32,
                          kind="ExternalOutput")

    with TileContext(nc) as tc:
        with (
            tc.tile_pool(name="fpool", bufs=2) as fpool,
            tc.tile_pool(name="wpool", bufs=3) as wpool,
            tc.tile_pool(name="apool", bufs=2) as apool,
            tc.tile_pool(name="tpool", bufs=4) as tpool,
        ):
            FI = 2 * NBLK * FCOLS  # free elems per i-shift slice
            for img in range(IMGS_PER_CORE):
                # one tile+DMA per i-shift so taps of i=0 start after ~1MB
                fvs = []
                for i in range(K):
                    f_t = fpool.tile([128, FI], f16, tag=f"f{i}")
                    nc.sync.dma_start(out=f_t[:],
                                      in_=fin[img][:, i * FI:(i + 1) * FI])
                    fvs.append(f_t[:].rearrange(
                        "p (par blk c) -> p par blk c",
                        par=2, blk=NBLK, c=FCOLS))

                acc_v = apool.tile([128, O_FREE], f16, tag="accv")
                acc_g = apool.tile([128, O_FREE], f16, tag="accg")
                av = acc_v[:].rearrange("p (blk c) -> p blk c", blk=NBLK)
                ag = acc_g[:].rearrange("p (blk c) -> p blk c", blk=NBLK)
                first = {id(av): True, id(ag): True}

                for tg in range(K):
                    w_t = wpool.tile([128, W_FREE], f16)
                    nc.sync.dma_start(out=w_t[:], in_=win[img, tg])
                    wv = w_t[:].rearrange("p (k blk c) -> p k blk c",
                                          k=K, blk=NBLK, c=W)
                    # gpsimd taps first: it has the worst per-op latency
                    korder = sorted(range(K),
                                    key=lambda k: tg * K + k not in GP_TAPS)
                    for k in korder:
                        t = tg * K + k
                        i, j = divmod(t, K)
                        par = j & 1
                        joff = j + par
                        f_ap = fvs[i][:, par, :, joff:joff + W]
                        w_ap = wv[:, k]
                        if t in GP_TAPS:
                            eng, acc, tag = nc.gpsimd, ag, "tmpg"
                        else:
                            eng, acc, tag = nc.vector, av, "tmpv"
                        if first[id(acc)]:
                            eng.tensor_mul(out=acc, in0=w_ap, in1=f_ap)
                            first[id(acc)] = False
                        else:
                            tmp = tpool.tile([128, O_FREE], f16, tag=tag)
                            tv = tmp[:].rearrange("p (blk c) -> p blk c",
                                                  blk=NBLK)
                            eng.tensor_mul(out=tv, in0=w_ap, in1=f_ap)
                            eng.tensor_add(out=acc, in0=acc, in1=tv)

                nc.vector.tensor_add(out=acc_v[:], in0=acc_v[:], in1=acc_g[:])
                # SWDGE cast fp16 -> fp32 on the store
                nc.gpsimd.dma_start(out=oout[img], in_=acc_v[:])
    nc.finalize()
    return nc


def _host_prep(frames, core):
    """Build per-core in_maps. frames [4,4,1,512,512] f32, core [4,4,25,1,512,512]."""
    G = NCORES * IMGS_PER_CORE  # 16
    F = np.ascontiguousarray(frames.reshape(G, H, W))
    Wc = core.reshape(G, K * K, H, W)

    # frames: pad rows 2/2, cols 3/4; Fp[g, r, c] = F[g, r-2, c-3]
    Fp = np.pad(F, ((0, 0), (2, 2), (3, 4))).astype(np.float16)
    fprep = np.empty((G, 128, K, 2, NBLK, FCOLS), np.float16)
    for i in range(K):
        for par in range(2):
            sl = Fp[:, i:i + H, (1 - par):(1 - par) + FCOLS]  # [G,512,518]
            fprep[:, :, i, par, :, :] = (
                sl.reshape(G, NBLK, 128, FCOLS).transpose(0, 2, 1, 3))

    # weights: [g][tg][p][k][blk][c]
    w16 = Wc.astype(np.float16)
    wprep = np.ascontiguousarray(
        w16.reshape(G, K, K, NBLK, 128, W).transpose(0, 1, 4, 2, 3, 5))

    in_maps = []
    for c in range(NCORES):
        g0 = c * IMGS_PER_CORE
        in_maps.append({
            "fin": np.ascontiguousarray(
                fprep[g0:g0 + IMGS_PER_CORE].reshape(IMGS_PER_CORE, 128, F_FREE)),
            "win": np.ascontiguousarray(
                wprep[g0:g0 + IMGS_PER_CORE].reshape(IMGS_PER_CORE, K, 128, W_FREE)),
        })
    return in_maps


def kernel(frames, core, bias):
    global last_results
    from concourse.bass_utils import run_bass_kernel_spmd

    frames = np.asarray(frames, dtype=np.float32)
    core = np.asarray(core, dtype=np.float32)

    if "nc" not in _compiled:
        _compiled["nc"] = _build_nc()
    nc = _compiled["nc"]

    in_maps = _host_prep(frames, core)
    trace = os.environ.get("KC_TRACE") == "1"
    tmpdir = os.environ.get("KC_TRACE_DIR") or None
    if tmpdir:
        os.makedirs(tmpdir, exist_ok=True)
    res = run_bass_kernel_spmd(nc, in_maps, list(range(NCORES)), trace=trace,
                               tmpdir=tmpdir)
    last_results = res

    G = NCORES * IMGS_PER_CORE
    out = np.empty((G, H, W), np.float32)
    for c in range(NCORES):
        o = res.results[c]["oout"]  # [2, 128, 2048] f32
        for img in range(IMGS_PER_CORE):
            out[c * IMGS_PER_CORE + img] = (
                o[img].reshape(128, NBLK, W).transpose(1, 0, 2).reshape(H, W))
    return out.reshape(4, 4, H, W)



# revision 3
# speedup vs baseline: 2.1734x; 2.1734x over previous
"""Per-pixel adaptive 5x5 conv (KPN) for Trainium2, 8-core data parallel.

out[g,h,w] = sum_{i,j} core[g,5i+j,h,w] * frames_pad[g,h+i-2,w+j-2]
with g = flattened (B,N) = 16 image planes; 2 planes per NeuronCore.

v2 design — PE-accumulate, DVE-multiply, no GpSimd compute:
  Row layout: partition p owns output rows 4p..4p+3; fin stores the 8-row
  halo (4p-2..4p+5) x 518 padded cols per partition (x2 replication only).
  Weights are host-shifted into a 518-wide y-grid per tap (w at y=x+j) so
  every DVE read starts 4B-aligned: tap (i,j) product
     q[p,r,y] = wsh[p,t,r,y] * f[p,r+i,y]          (one fp16 TT mul, 2x mode)
  and out row r: acc[x] += q[p,r,x+j] is summed by the IDLE TensorEngine as
  an identity-stationary matmul accumulating 25 taps into PSUM bank r
  (fp32), moving slice taken at free offset r*518+j (PE needs no alignment).
  ACT evacuates PSUM->SBUF with the fp32->fp16 cast; HWDGE does all DMAs.
  GpSimd does nothing: its SBUF port contends with DVE tensor_tensor ops.
"""

import os
import sys
from contextlib import ExitStack

import numpy as np

for _p in ("/opt/trn_rl_repo",):
    if _p not in sys.path and os.path.isdir(_p):
        sys.path.insert(0, _p)

K = 5
NCORES = 8
IMGS_PER_CORE = 2
H = W = 512
RPP = 4            # output rows per partition
FROWS = 8          # stored halo rows per partition
YC = 518           # padded column grid
F_FREE = FROWS * YC          # 4144
WG_FREE = K * RPP * YC       # 10360 (one 5-tap group)
Q_FREE = RPP * YC            # 2072 (product tile)
O_FREE = RPP * W             # 2048

_compiled = {}
last_results = None  # BassKernelResults of the most recent run (for test.py)


def _build_nc():
    import concourse.bacc as bacc
    import concourse.mybir as mybir
    from concourse.masks import make_identity
    from concourse.tile import TileContext

    f16 = mybir.dt.float16
    f32 = mybir.dt.float32

    nc = bacc.Bacc(None, target_bir_lowering=False, debug=False)
    fin = nc.dram_tensor("fin", [IMGS_PER_CORE, 128, F_FREE], f16,
                         kind="ExternalInput")
    win = nc.dram_tensor("win", [IMGS_PER_CORE, K, 128, WG_FREE], f16,
                         kind="ExternalInput")
    oout = nc.dram_tensor("oout", [IMGS_PER_CORE, 128, O_FREE], f16,
                          kind="ExternalOutput")

    ctx = ExitStack()
    with TileContext(nc) as tc:
        with (
            tc.tile_pool(name="const", bufs=1) as cpool,
            tc.tile_pool(name="fpool", bufs=2) as fpool,
            tc.tile_pool(name="wpool", bufs=3) as wpool,
            tc.tile_pool(name="tpool", bufs=6) as tpool,
            tc.tile_pool(name="opool", bufs=2) as opool,
            tc.tile_pool(name="psum", bufs=2, space="PSUM") as ppool,
        ):
            ident = cpool.tile([128, 128], f16)
            make_identity(nc, ident[:])

            for img in range(IMGS_PER_CORE):
                f_t = fpool.tile([128, F_FREE], f16, tag="f")
                nc.sync.dma_start(out=f_t[:], in_=fin[img])
                fv = f_t[:].rearrange("p (r y) -> p r y", r=FROWS, y=YC)

                ps = ppool.tile([128, O_FREE], f32, tag="ps")

                for tg in range(K):
                    w_t = wpool.tile([128, WG_FREE], f16)
                    nc.sync.dma_start(out=w_t[:], in_=win[img, tg])
                    wv = w_t[:].rearrange("p (k r y) -> p k r y",
                                          k=K, r=RPP, y=YC)
                    for k in range(K):
                        t = tg * K + k
                        i, j = divmod(t, K)
                        tmp = tpool.tile([128, Q_FREE], f16)
                        tv = tmp[:].rearrange("p (r y) -> p r y",
                                              r=RPP, y=YC)
                        nc.vector.tensor_mul(out=tv, in0=wv[:, k],
                                             in1=fv[:, i:i + RPP, :])
                        for b in range(RPP):
                            nc.tensor.matmul(
                                ps[:, b * W:(b + 1) * W],
                                ident[:],
                                tmp[:, b * YC + j: b * YC + j + W],
                                start=(t == 0),
                                stop=(t == K * K - 1),
                            )

                o_t = opool.tile([128, O_FREE], f16, tag="o")
                nc.scalar.activation(
                    out=o_t[:], in_=ps[:],
                    func=mybir.ActivationFunctionType.Copy)
                nc.sync.dma_start(out=oout[img], in_=o_t[:])
    nc.finalize()
    ctx.close()
    return nc


def _host_prep(frames, core):
    """Build per-core in_maps. frames [4,4,1,512,512] f32, core [4,4,25,1,512,512]."""
    G = NCORES * IMGS_PER_CORE  # 16
    F = np.ascontiguousarray(frames.reshape(G, H, W))
    Wc = core.reshape(G, K * K, H, W)

    # frames: pad rows 2/2, cols 2/4 -> [G, 516, 518]; stored row rr at
    # partition p = image row 4p-2+rr = padded row 4p+rr.
    Fp = np.pad(F, ((0, 0), (2, 2), (2, 4))).astype(np.float16)
    rows = 4 * np.arange(128)[:, None] + np.arange(FROWS)[None, :]  # [128,8]
    fprep = Fp[:, rows, :]  # [G, 128, 8, 518]

    # weights: shift tap (i,j) into the y-grid at y = x + j, zero elsewhere.
    w16 = Wc.astype(np.float16)
    wsh = np.zeros((G, K * K, H, YC), np.float16)
    for t in range(K * K):
        j = t % K
        wsh[:, t, :, j:j + W] = w16[:, t]
    # [g, t, 4p+r, y] -> [g, tg, p, k, r, y]
    wprep = np.ascontiguousarray(
        wsh.reshape(G, K, K, 128, RPP, YC).transpose(0, 1, 3, 2, 4, 5))

    in_maps = []
    for c in range(NCORES):
        g0 = c * IMGS_PER_CORE
        in_maps.append({
            "fin": np.ascontiguousarray(
                fprep[g0:g0 + IMGS_PER_CORE].reshape(
                    IMGS_PER_CORE, 128, F_FREE)),
            "win": np.ascontiguousarray(
                wprep[g0:g0 + IMGS_PER_CORE].reshape(
                    IMGS_PER_CORE, K, 128, WG_FREE)),
        })
    return in_maps


def kernel(frames, core, bias):
    global last_results
    from concourse.bass_utils import run_bass_kernel_spmd

    frames = np.asarray(frames, dtype=np.float32)
    core = np.asarray(core, dtype=np.float32)

    if "nc" not in _compiled:
        _compiled["nc"] = _build_nc()
    nc = _compiled["nc"]

    in_maps = _host_prep(frames, core)
    trace = os.environ.get("KC_TRACE") == "1"
    tmpdir = os.environ.get("KC_TRACE_DIR") or None
    if tmpdir:
        os.makedirs(tmpdir, exist_ok=True)
    res = run_bass_kernel_spmd(nc, in_maps, list(range(NCORES)), trace=trace,
                               tmpdir=tmpdir)
    last_results = res

    G = NCORES * IMGS_PER_CORE
    out = np.empty((G, H, W), np.float32)
    for c in range(NCORES):
        o = res.results[c]["oout"]  # [2, 128, 2048] f16
        for img in range(IMGS_PER_CORE):
            out[c * IMGS_PER_CORE + img] = (
                o[img].astype(np.float32).reshape(H, W))
    return out.reshape(4, 4, H, W)
